# revision 2
# baseline (speedup 1.0000x reference)
"""ConvPMF forward on 8 Trainium2 NeuronCores (Bass/Tile).

Per core (data-parallel over the valid (batch, review) pairs):
  1. host packs this core's review words into a flat stream and uploads
     embed[words] as a bf16 DRAM table in stream order; the device pulls it
     in with dma_gather(transpose=True) (xbar-transpose SWDGE, 512 ids per
     instruction = the ring-carveout limit), which lands each 1024-word
     group directly as the conv rhs layout [128 dim, 1024 words] in SBUF —
     no PE transposes, no PSUM->SBUF copies.
  2. Conv1d(SAME) as 5 PSUM-accumulated matmuls with shifted/clipped rhs
     windows, bf16 weights x bf16 rev (1 cycle/row on the PE vs fp32's 4);
     4 reviews run concurrently on the tensor engine via column tiling
     (tile_position, M=32 each), fp32 PSUM accumulation.
  3. max-softmax pool:  max(softmax(fm)) == 1 / sum_w exp(fm - max_w fm)
     -> reduce_max (DVE), Exp with accum_out (ACT), reciprocal (DVE)
Host: shard the ragged review list, combine pooled vectors into item
embeddings, dot with user factors, add bias.
"""
import math

import ml_dtypes
import numpy as np

import concourse.bass as bass
import concourse.mybir as mybir
import concourse.tile as tile
from concourse import bacc
from concourse.bass_utils import run_bass_kernel_spmd

f32 = mybir.dt.float32
bf16 = mybir.dt.bfloat16
i16 = mybir.dt.int16

N_CORES = 8
D, F, K = 128, 32, 5           # embed dim, factors (conv out channels), window
W = 256                        # words per review
GCH = 512                      # ids per dma_gather (ring carveout caps ~64
                               # descs/dma; 1024 ids hangs the Q7 descgen)
TAP_ORDER = (2, 0, 1, 3, 4)    # tap 2 covers full width -> start=True first

_program_cache: dict[int, bass.Bass] = {}


def _build_program(groups: int) -> bass.Bass:
    """One SPMD program, identical on all cores: `groups` groups of 4
    reviews, review slot j of a group on tensor-engine column group j."""
    nwords = groups * 4 * W

    nc = bacc.Bacc("TRN2", target_bir_lowering=False, debug=False)
    table_d = nc.dram_tensor("table", [nwords, D], bf16, kind="ExternalInput")
    idx_d = nc.dram_tensor("idx", [128, GCH // 16], i16, kind="ExternalInput")
    cst_d = nc.dram_tensor("cst", [128, K * F], bf16, kind="ExternalInput")
    pooled_d = nc.dram_tensor("pooled", [128, groups], f32, kind="ExternalOutput")

    with tile.TileContext(nc) as tc:
        with tc.tile_pool(name="const", bufs=1) as cpool, \
             tc.tile_pool(name="gat", bufs=4) as gpool, \
             tc.tile_pool(name="wrk", bufs=2) as wpool, \
             tc.tile_pool(name="psF", bufs=2, space="PSUM") as fmpool:
            idx_sb = cpool.tile([128, GCH // 16], i16)
            nc.sync.dma_start(idx_sb[:], idx_d[:])
            cst_sb = cpool.tile([128, K * F], bf16)
            nc.sync.dma_start(cst_sb[:], cst_d[:])
            pooled_sb = cpool.tile([128, groups], f32)

            def wk(k):
                return cst_sb[:, k * F:(k + 1) * F]

            for g in range(groups):
                g_sb = gpool.tile([128, 1, 4 * W], bf16, tag="g")
                for h in range(4 * W // GCH):
                    nc.gpsimd.dma_gather(
                        out_ap=g_sb[:, :, h * GCH:(h + 1) * GCH],
                        in_ap=table_d[g * 4 * W + h * GCH:
                                      g * 4 * W + (h + 1) * GCH, :],
                        idxs_ap=idx_sb[:],
                        num_idxs=GCH, num_idxs_reg=GCH,
                        elem_size=D, transpose=True)

                fm_ps = fmpool.tile([128, W], f32, tag="fm")
                for k in TAP_ORDER:
                    s = k - 2                     # word shift of this tap
                    a, ob = max(0, s), max(0, -s)
                    n = W - abs(s)
                    for j in range(4):
                        nc.tensor.matmul(
                            fm_ps[32 * j:32 * j + 32, ob:ob + n],
                            lhsT=wk(k), rhs=g_sb[:, 0, j * W + a:j * W + a + n],
                            start=(k == TAP_ORDER[0]), stop=(k == TAP_ORDER[-1]),
                            tile_position=(0, 32 * j))

                negm = wpool.tile([128, 1], f32, tag="negm")
                nc.vector.tensor_reduce(
                    negm[:], fm_ps[:], axis=mybir.AxisListType.X,
                    op=mybir.AluOpType.max, negate=True)
                e_sb = wpool.tile([128, W], f32, tag="e")
                ssum = wpool.tile([128, 1], f32, tag="ssum")
                nc.scalar.activation(
                    e_sb[:], fm_ps[:], mybir.ActivationFunctionType.Exp,
                    bias=negm[:], scale=1.0, accum_out=ssum[:])
                nc.vector.reciprocal(pooled_sb[:, g:g + 1], ssum[:])
                nc.sync.dma_start(pooled_d[:, g:g + 1], pooled_sb[:, g:g + 1])
    nc.compile()
    return nc


def prepare(user_indices, docs, review_counts, w_user, embed_matrix,
            conv_weight, bias):
    """Host-side sharding prep: returns (nc, in_maps, valid, n_core) or None
    when there are no valid reviews."""
    docs = np.asarray(docs)
    review_counts = np.asarray(review_counts)
    embed_bf = np.asarray(embed_matrix, dtype=ml_dtypes.bfloat16)
    conv_weight = np.asarray(conv_weight, dtype=np.float32)

    b_sz = docs.shape[0]
    valid = [(b, r) for b in range(b_sz) for r in range(int(review_counts[b]))]
    if not valid:
        return None

    n_core = math.ceil(len(valid) / N_CORES)
    groups = math.ceil(n_core / 4)
    n_core = groups * 4
    nwords = groups * 4 * W

    cst = np.zeros((128, K * F), dtype=ml_dtypes.bfloat16)
    for k in range(K):
        cst[:, k * F:(k + 1) * F] = conv_weight[:, :, k].T.astype(
            ml_dtypes.bfloat16)

    # identity gather ids for one 512-word chunk, wrapped 16-partition
    # snake, replicated to all 128 partitions (each SWDGE queue/cpu pair
    # reads its own 16-channel window)
    idx16 = np.zeros((16, GCH // 16), dtype=np.int16)
    for j in range(GCH):
        idx16[j % 16, j // 16] = j
    idx = np.tile(idx16, (8, 1))

    docs32 = docs.astype(np.int64)
    in_maps = []
    for c in range(N_CORES):
        words = np.zeros((nwords,), dtype=np.int64)
        base = c * n_core
        take = min(n_core, len(valid) - base)
        for slot in range(max(take, 0)):
            bb, rr = valid[base + slot]
            words[slot * W:(slot + 1) * W] = docs32[bb, rr]
        table = np.ascontiguousarray(embed_bf[words])
        in_maps.append({"table": table, "idx": idx, "cst": cst})

    nc = _program_cache.get(groups)
    if nc is None:
        nc = _build_program(groups)
        _program_cache[groups] = nc
    return nc, in_maps, valid, n_core


def kernel(user_indices, docs, review_counts, w_user, embed_matrix, conv_weight,
           bias):
    user_indices = np.asarray(user_indices)
    docs = np.asarray(docs)
    review_counts = np.asarray(review_counts)
    w_user = np.asarray(w_user, dtype=np.float32)
    conv_weight = np.asarray(conv_weight, dtype=np.float32)
    bias = np.asarray(bias, dtype=np.float32)

    b_sz = docs.shape[0]
    denom = np.maximum(review_counts, 1).astype(np.float32)
    prep = prepare(user_indices, docs, review_counts, w_user, embed_matrix,
                   conv_weight, bias)
    if prep is None:
        return np.full((b_sz,), bias[0], dtype=np.float32)
    nc, in_maps, valid, n_core = prep

    res = run_bass_kernel_spmd(nc, in_maps, list(range(N_CORES)))

    item = np.zeros((b_sz, F), dtype=np.float32)
    for i, (bb, rr) in enumerate(valid):
        c, slot = i // n_core, i % n_core
        g, j = slot // 4, slot % 4
        item[bb] += res.results[c]["pooled"][32 * j:32 * j + 32, g]
    item /= denom[:, None]
    out = (w_user[user_indices] * item).sum(axis=-1) + bias[0]
    return out.astype(np.float32)
